# revision 53
# baseline (speedup 1.0000x reference)
"""Trainium2 Bass kernel for a GPT-style transformer block (B=2, T=2048, C=1024, H=16).

Sharding: Megatron-style tensor parallelism over 8 NeuronCores.
  - Attention is head-parallel: each core computes QKV / attention for its 2 heads
    over all 4096 tokens.
  - Per batch, a small fp8 AllToAll (256 KB/core) redistributes unnormalized-exp
    attention outputs from head-sharded to token-sharded.
  - The output projection, LayerNorm2 and the MLP run token-parallel on a merged
    512-token shard per core (256 tokens of each batch) so FC/MP matmuls stream
    N=512 moving operands.

fp8 (e4m3, TRN max +-240) with DoubleRow double-pumping is used for the GEMMs
where quantization noise is cheap: QKV projection, LN stats, attention AV, and
the attention output projection.  FC and MP stay bf16 (fp8 there costs ~1.7%
rel err each -- over the 2e-2 budget).  Weights for fp8 GEMMs are pre-scaled
x16 on the host to dodge e4m3 subnormals; the descale rides for free on the
existing per-token affine multiplies (rstd/16) and stt scalars.  y ships x8.

Layouts are "transposed" [feature, token].  LN affines fold into weights on the
host; LN1 normalization folds around the QKV matmul as a per-token affine
(rank-1 wsum x nmu subtracts the mean inside the PSUM group).  rstd comes from
exp(-0.5*ln(var+eps)) so ScalarE stays inside the ln/exp table set the softmax
needs; squares run on VectorE only (no Square-table thrash).  Row stats are
broadcast across partitions with GpSimd stride-0 DMAs off a DRAM bounce;
softmax denominators ride a ones-column in the V stationary; V tiles are
PE-transposed.  Collectives and the phase-D weight prefetch issue on the DVE
queue so they never block the GpSimd broadcast queue.
"""

from contextlib import ExitStack

import numpy as np
import ml_dtypes

import concourse.bass as bass
import concourse.bacc as bacc
import concourse.mybir as mybir
import concourse.tile as tile
from concourse.bass_utils import run_bass_kernel_spmd

BF16 = mybir.dt.bfloat16
FP8 = mybir.dt.float8e4
F32 = mybir.dt.float32
AF = mybir.ActivationFunctionType
OP = mybir.AluOpType
DR = mybir.MatmulPerfMode.DoubleRow

N_CORES = 8
B, T, C, H, D = 2, 2048, 1024, 16, 64
NTOK = B * T  # 4096
F = 4 * C  # 4096
LN_EPS = 1e-5
HPC = H // N_CORES  # heads per core = 2
HALF = 256  # phase-D tokens per batch per core
NCH = C // 128  # 8 channel blocks
NCP = NCH // 2  # 4 channel pairs (DoubleRow)
NFB = F // 128  # 32 ffn blocks
NCHUNK = NTOK // 512  # 8 token chunks of 512
SB = 128  # s-block size
VWP = 80  # padded V block width (64 V + 1 ones + 15 pad; 16-aligned)
WS = 16.0  # fp8 weight pre-scale
YS = 8.0  # fp8 y pre-scale
MTS = 4.0  # fp8 gelu-output pre-scale
LOG_WS = float(np.log(WS))

_CACHE = {}

_UID = [0]


def _t(pool, shape, dtype, tag):
    _UID[0] += 1
    return pool.tile(shape, dtype, tag=tag, name=f"{tag}_{_UID[0]}")


def _pbc(ap, n):
    """Partition-broadcast AP: read `ap` n times across partitions."""
    return bass.AP(tensor=ap.tensor, offset=ap.offset,
                   ap=[[0, n]] + [list(x) for x in ap.ap])


def _build(with_bias_qkv: bool, with_bias_ao: bool,
           with_bias_mp: bool):
    nc = bacc.Bacc("TRN2", target_bir_lowering=False, debug=False,
                   num_devices=N_CORES)

    # ---- I/O ----
    xT_d = nc.dram_tensor("xT", [NCHUNK, 128, NCP, 2, 512], FP8,
                          kind="ExternalInput")
    xTs_d = nc.dram_tensor("xTs", [128, NCH, 2 * HALF], F32,
                           kind="ExternalInput")
    wqkv_d = nc.dram_tensor("wqkv", [128, 3, NCP, 2, 128], FP8,
                            kind="ExternalInput")
    wsum_d = nc.dram_tensor("wsum", [1, 3 * 128], BF16, kind="ExternalInput")
    bqkv_d = nc.dram_tensor("bqkv", [1, 3 * 128], BF16, kind="ExternalInput")
    wao_d = nc.dram_tensor("wao", [128, NCP, 2, NCH, 128], FP8,
                           kind="ExternalInput")
    bao_d = nc.dram_tensor("bao", [128, NCH], F32, kind="ExternalInput")
    wfc_d = nc.dram_tensor("wfc", [NFB // 4, 128, NCH, 4, 128], BF16,
                           kind="ExternalInput")
    bfc_d = nc.dram_tensor("bfc", [128, NFB], F32, kind="ExternalInput")
    wmp_d = nc.dram_tensor("wmp", [NCH, 128, NFB // 2, 2, 128], FP8,
                           kind="ExternalInput")
    bmp_d = nc.dram_tensor("bmp", [128, NCH], F32, kind="ExternalInput")
    mask_d = nc.dram_tensor("mask", [128, 128], FP8, kind="ExternalInput")
    ident_d = nc.dram_tensor("ident", [128, 128], BF16, kind="ExternalInput")
    out_d = nc.dram_tensor("out", [C, 2 * HALF], F32, kind="ExternalOutput")

    with tile.TileContext(nc) as tc, ExitStack() as _es:
        singles = _es.enter_context(tc.tile_pool(name="singles", bufs=1))
        dram = _es.enter_context(tc.tile_pool(name="dram", bufs=1, space="DRAM"))
        resid = _es.enter_context(tc.tile_pool(name="resid", bufs=1))
        pool_ya = _es.enter_context(tc.tile_pool(name="ya", bufs=4))
        pool_w3 = _es.enter_context(tc.tile_pool(name="w3", bufs=3))
        pool_ao = _es.enter_context(tc.tile_pool(name="ao", bufs=4))

        # DRAM bounce rows for broadcasts; A2A buffers (fp8)
        arow_d = _t(dram, [NCHUNK, 512], F32, "arow_d")
        den_d = _t(dram, [NCHUNK, HPC, 512], F32, "den_d")
        a2a_in1 = _t(dram, [N_CORES, HPC, D, HALF], FP8, "a2a_in1")
        a2a_out1 = _t(dram, [N_CORES, HPC, D, HALF], FP8, "a2a_out1")
        a2a_in2 = _t(dram, [N_CORES, HPC, D, HALF], FP8, "a2a_in2")
        a2a_out2 = _t(dram, [N_CORES, HPC, D, HALF], FP8, "a2a_out2")

        with ExitStack() as es1:
            # PSUM: psq 2 banks, psc 2x2 banks, psy 2 banks = 8
            psq = es1.enter_context(tc.tile_pool(name="psq", bufs=2,
                                                 space="PSUM"))
            pscp = es1.enter_context(tc.tile_pool(name="ps_c", bufs=2,
                                                  space="PSUM"))
            psyp = es1.enter_context(tc.tile_pool(name="ps_y", bufs=2,
                                                  space="PSUM"))
            pool_xt = es1.enter_context(tc.tile_pool(name="xt", bufs=NCHUNK))
            pool_sqx = es1.enter_context(tc.tile_pool(name="sqx", bufs=3))
            pool_row = es1.enter_context(tc.tile_pool(name="row", bufs=3))
            pool_bc = es1.enter_context(tc.tile_pool(name="bc", bufs=3))
            pool_vt = es1.enter_context(tc.tile_pool(name="vt", bufs=2))
            pool_att = es1.enter_context(tc.tile_pool(name="att", bufs=4))
            # enough yt buffers that batch-1 stores can queue behind A2A#1
            pool_yt = es1.enter_context(tc.tile_pool(name="yt", bufs=8))
            pool_qkv = es1.enter_context(tc.tile_pool(name="qkv", bufs=1))

            xt_tiles = {}

            def load_chunk(g):
                t0 = 512 * g
                xt = _t(pool_xt, [128, NCP, 2, 512], FP8, "xt")
                eng = nc.sync if g in (1, 3, 5, 7) else nc.scalar
                eng.dma_start(xt[:], xT_d[g])
                xt_tiles[g] = xt

            # chunk 0 first so PE can start ASAP, then weights, then rest
            load_chunk(0)

            ones_bf = _t(singles, [128, 1], BF16, "ones_bf")
            nc.vector.memset(ones_bf[:], 1.0)
            ones2 = _t(singles, [128, 2, 16], FP8, "ones2")
            nc.vector.memset(ones2[:], 1.0)
            onesrow_bf = _t(singles, [1, 128], BF16, "onesrow_bf")
            nc.vector.memset(onesrow_bf[:], 1.0)
            onesrow_f32 = _t(singles, [1, 64], F32, "onesrow_f32")
            nc.vector.memset(onesrow_f32[:], 1.0)
            eps_t = _t(singles, [128, 1], F32, "eps")
            nc.vector.memset(eps_t[:], LN_EPS)
            nlws_t = _t(singles, [128, 1], F32, "nlws")
            nc.vector.memset(nlws_t[:], -LOG_WS)

            ao_b0 = [_t(pool_ao, [128, 2, 512], BF16, "aop")
                     for _ in range(NCP)]
            wqkv_t = _t(resid, [128, 3, NCP, 2, 128], FP8, "wqkv")
            nc.sync.dma_start(wqkv_t[:], wqkv_d[:])
            wsum_t = _t(resid, [1, 384], BF16, "wsum")
            nc.sync.dma_start(wsum_t[:], wsum_d[:])

            load_chunk(1)

            # phase-D resident tiles; DMAs staggered into the chunk loop on
            # the Scalar queue so no single queue stalls at startup
            wao_t = _t(resid, [128, NCP, 2, NCH, 128], FP8, "wao")
            xts_t = _t(resid, [128, NCH, 2 * HALF], F32, "xts")
            bfc_t = _t(singles, [128, NFB], F32, "bfc")
            bmp_t = _t(singles, [128, NCH], F32, "bmp")
            bao_t = _t(singles, [128, NCH], F32, "bao")

            def prefetch(step):
                if step < 0:
                    return
                if step == 0:
                    nc.scalar.dma_start(wao_t[:], wao_d[:])
                elif step == 1:
                    nc.scalar.dma_start(xts_t[:], xTs_d[:])
                elif step == 2:
                    nc.scalar.dma_start(bfc_t[:], bfc_d[:])
                    nc.scalar.dma_start(bmp_t[:], bmp_d[:])
                    if with_bias_ao:
                        nc.scalar.dma_start(bao_t[:], bao_d[:])

            for g in (2, 3, 4, 6):
                load_chunk(g)

            mask_t = _t(resid, [128, 128], FP8, "mask")
            nc.sync.dma_start(mask_t[:], mask_d[:])
            ident_t = _t(resid, [128, 128], BF16, "ident")
            nc.sync.dma_start(ident_t[:], ident_d[:])
            bqkv_col_t = _t(resid, [128, 3], F32, "bqkv_col")
            if with_bias_qkv:
                nc.gpsimd.dma_start(
                    bqkv_col_t[:],
                    bqkv_d.ap()[0, :].rearrange("(o p) -> p o", p=128))

            # attention operand tiles (resident, written per chunk)
            qT_t = _t(pool_qkv, [128, NTOK], BF16, "qT")
            kT_t = _t(pool_qkv, [128, NTOK], BF16, "kT")
            vT_t = _t(pool_qkv, [128, NTOK], BF16, "vT")

            sq_tiles = {}

            def squares(g):
                xt = xt_tiles[g]
                sq = _t(pool_sqx, [128, NCP, 2, 512], FP8, "sqx")
                nc.vector.tensor_tensor(sq[:, 0:2], xt[:, 0:2], xt[:, 0:2],
                                        OP.mult)
                nc.vector.tensor_tensor(sq[:, 2:4], xt[:, 2:4], xt[:, 2:4],
                                        OP.mult)
                sq_tiles[g] = sq

            ab_tiles = {}

            def stats_chunk(g):
                """s1 = sum_c x (row 0), s2 = sum_c x^2 (row 32) in one PSUM
                bank via DoubleRow fp8; then the row chain + abc broadcast."""
                xt = xt_tiles[g]
                sq = sq_tiles.pop(g)
                sp = _t(psq, [64, 512], F32, "psq")
                for i in range(NCP):
                    nc.tensor.matmul(sp[0:1, :], ones2[:, :, 0:1],
                                     xt[:, i, :, :], perf_mode=DR,
                                     start=(i == 0), stop=(i == NCP - 1))
                # DoubleRow cannot target PSUM base partition 32, so the
                # s2 chain runs as plain fp8 matmuls (bf16 rate)
                for i in range(NCP):
                    for jj in range(2):
                        nc.tensor.matmul(sp[32:33, :], ones2[:, 0, 0:1],
                                         sq[:, i, jj, :],
                                         start=False,
                                         stop=(i == NCP - 1 and jj == 1),
                                         skip_group_check=True)
                nmu = _t(pool_row, [1, 512], BF16, "nmu")
                nc.vector.tensor_scalar(out=nmu[:], in0=sp[0:1, :],
                                        scalar1=-1.0 / C, scalar2=0.0,
                                        op0=OP.mult, op1=OP.add)
                # varp = s2 - s1^2/C  (= var*C); squares on DVE, no Act table
                # (s1 * nmu = -s1^2/C; only one PSUM operand allowed)
                varp = _t(pool_row, [1, 512], F32, "varp")
                nc.vector.scalar_tensor_tensor(
                    out=varp[:], in0=sp[0:1, :], scalar=1.0,
                    in1=nmu[:], op0=OP.mult, op1=OP.mult)
                varq = _t(pool_row, [1, 512], F32, "varq")
                nc.vector.tensor_tensor(varq[:], varp[:], sp[32:33, :], OP.add)
                # rstd/WS = exp(-0.5*ln(varp/C + eps) - ln WS)
                lnv = _t(pool_row, [1, 512], F32, "lnv")
                nc.scalar.activation(lnv[:], varq[:], AF.Ln,
                                     bias=eps_t[0:1, :], scale=1.0 / C)
                arow = _t(pool_row, [1, 512], F32, "arow")
                nc.scalar.activation(arow[:], lnv[:], AF.Exp, scale=-0.5,
                                     bias=nlws_t[0:1, :])
                nc.sync.dma_start(arow_d[g], arow[0:1, :])
                abc = _t(pool_bc, [128, 512], F32, "abc")
                nc.sync.dma_start(abc[:], _pbc(arow_d[g], 128))
                ab_tiles[g] = (abc, nmu)

            def qkv_piece(g, o, dst_t):
                """One projection (Q, K or V) of chunk g -- a ~1us PE filler
                sized to hide one softmax-exp latency window."""
                def run():
                    t0 = 512 * g
                    xt = xt_tiles[g]
                    abc, nmu = ab_tiles[g]
                    ps = _t(psq, [128, 512], F32, "psq")
                    for i in range(NCP):
                        nc.tensor.matmul(ps[:], wqkv_t[:, o, i, :, :],
                                         xt[:, i, :, :], perf_mode=DR,
                                         start=(i == 0), stop=False)
                    # rank-1 term: wsum (x) nmu subtracts the mean
                    nc.tensor.matmul(ps[:], wsum_t[:, 128 * o:128 * (o + 1)],
                                     nmu[:], start=False, stop=True,
                                     skip_group_check=True)
                    dst = dst_t[:, t0:t0 + 512]
                    if o == 0:  # fold 1/sqrt(D) into Q
                        nc.vector.scalar_tensor_tensor(
                            out=dst, in0=ps[:], scalar=0.125, in1=abc[:],
                            op0=OP.mult, op1=OP.mult)
                    else:
                        nc.vector.tensor_tensor(dst, ps[:], abc[:], OP.mult)
                    if with_bias_qkv:
                        nc.vector.tensor_scalar(
                            out=dst, in0=dst,
                            scalar1=bqkv_col_t[:, o:o + 1], scalar2=0.0,
                            op0=OP.add, op1=OP.add)
                    if o == 2:
                        xt_tiles.pop(g)
                        ab_tiles.pop(g)
                return run

            def qkv_fillers(g):
                return [qkv_piece(g, 0, qT_t), qkv_piece(g, 1, kT_t),
                        qkv_piece(g, 2, vT_t)]

            def qkv_chunk(g):
                for f in qkv_fillers(g):
                    f()

            vt = {}

            def att_chunk(b, j, fillers=()):
                fillers = list(fillers)
                g = 4 * b + j
                t0 = 512 * g
                if j == 0:
                    for h in range(HPC):
                        v = _t(pool_vt, [128, T // SB // 2, 2, VWP], FP8,
                               f"vt{h}")
                        nc.vector.memset(v[:, :, :, D:D + 1], 1.0)
                        nc.vector.memset(v[:, :, :, D + 1:VWP], 0.0)
                        vt[b, h] = v
                # transpose V for the newly available s-blocks
                for i in range(4 * j, 4 * j + 4):
                    s0 = 2048 * b + SB * i
                    for h in range(HPC):
                        pst = _t(psq, [128, D], BF16, "psq")
                        nc.tensor.transpose(
                            pst[:],
                            vT_t[64 * h:64 * (h + 1), s0:s0 + SB],
                            ident_t[64 * h:64 * (h + 1),
                                    64 * h:64 * (h + 1)])
                        nc.vector.tensor_copy(
                            vt[b, h][:, i // 2, i % 2, 0:D], pst[:])
                psy = [_t(psyp, [VWP, 512], F32, "psy") for h in range(HPC)]
                npair = 2 * j + 2
                pend_av = None
                for p in range(npair):
                    at = _t(pool_att, [128, 2, HPC, 512], FP8, "att")
                    for par in range(2):
                        i = 2 * p + par
                        m = i - 4 * j  # >= 0 on diagonal blocks
                        f0 = 128 * m if m >= 0 else 0
                        s0 = 2048 * b + SB * i
                        psc = _t(pscp, [128, HPC, 512], F32, "psc")
                        for h in range(HPC):
                            nc.tensor.matmul(
                                psc[:, h, 0:512 - f0],
                                kT_t[64 * h:64 * (h + 1), s0:s0 + SB],
                                qT_t[64 * h:64 * (h + 1), t0 + f0:t0 + 512],
                                tile_position=(64 * h, 0),
                            )
                        nc.scalar.activation(at[:, par, :, f0:512],
                                             psc[:, :, 0:512 - f0], AF.Exp)
                        if m >= 0:  # diagonal: mask boundary block
                            for h in range(HPC):
                                nc.vector.tensor_tensor(
                                    at[:, par, h, f0:f0 + 128],
                                    at[:, par, h, f0:f0 + 128],
                                    mask_t[:], OP.mult)
                    # zero the invalid wedge of the second diagonal block
                    pf0 = 256 if p == 2 * j + 1 else 0
                    mlast = 2 * p + 1 - 4 * j
                    if mlast >= 0:
                        w0 = 128 * (mlast - 1) if mlast > 0 else 0
                        nc.vector.memset(at[:, 1, :, w0:128 * mlast], 0.0)
                    # AV trails by one pair so the next pair's QK matmuls
                    # hide the softmax-exp latency instead of the PE stalling
                    if pend_av is not None:
                        pend_av()

                    def mk_av(p=p, at=at, pf0=pf0):
                        def run():
                            for h in range(HPC):
                                nc.tensor.matmul(
                                    psy[h][:, pf0:512],
                                    vt[b, h][:, p, :, :],
                                    at[:, :, h, pf0:512],
                                    perf_mode=DR,
                                    start=(p == 0), stop=(p == npair - 1),
                                    skip_group_check=True)
                        return run
                    pend_av = mk_av()
                pend_av()
                # normalize y on the producer, then ship 256-col halves
                a2a_in = a2a_in1 if b == 0 else a2a_in2
                rr = _t(pool_row, [VWP, HPC, 512], F32, "rr")
                for h in range(HPC):
                    nc.vector.tensor_copy(rr[0:1, h, :],
                                          psy[h][D:D + 1, :])
                    if not (b == 1 and j == 3):
                        nc.sync.dma_start(den_d[g, h], rr[0:1, h, :])
                d0 = 2 * (g % 4)
                for h in range(HPC):
                    if b == 1 and j == 3:
                        # last chunk: PE-broadcast the denominator so the
                        # A2A#2 entry skips the DRAM bounce roundtrip
                        rbp = _t(psq, [64, 512], F32, "psq")
                        nc.tensor.matmul(rbp[:], onesrow_f32[:],
                                         rr[0:1, h, :])
                        rec = _t(pool_bc, [64, 512], F32, "rec")
                        nc.vector.reciprocal_approx_fast(rec[:], rbp[:])
                    else:
                        rbc = _t(pool_bc, [64, 512], F32, "rbc")
                        nc.sync.dma_start(rbc[:], _pbc(den_d[g, h], 64))
                        rec = _t(pool_bc, [64, 512], F32, "rec")
                        nc.vector.reciprocal_approx_fast(rec[:], rbc[:])
                    yt = _t(pool_yt, [64, 512], FP8, "yt")
                    nc.vector.scalar_tensor_tensor(
                        out=yt[:], in0=psy[h][0:D, :], scalar=YS,
                        in1=rec[:], op0=OP.mult, op1=OP.mult)
                    nc.sync.dma_start(a2a_in[d0, h].opt(), yt[:, 0:HALF])
                    nc.sync.dma_start(a2a_in[d0 + 1, h].opt(),
                                      yt[:, HALF:512])

            # --- batch 0: stats run TWO chunks ahead of qkv so the
            # abc DRAM-bounce broadcast never gates the qkv post-multiply ---
            squares(0)
            stats_chunk(0)
            load_chunk(5)
            load_chunk(7)
            squares(1)
            stats_chunk(1)
            squares(2)
            for j in range(4):
                qkv_chunk(j)
                stats_chunk(j + 2)
                squares(j + 3)
                if j == 3:
                    squares(7)
                att_chunk(0, j)
            nc.gpsimd.collective_compute(
                "AllToAll", OP.bypass,
                replica_groups=[list(range(N_CORES))],
                ins=[a2a_in1.opt()], outs=[a2a_out1.opt()],
            )
            # start loading the batch-0 halves of the ya pair tiles ASAP
            ya_t = []
            for i in range(NCP):
                ya = _t(pool_ya, [128, 2, 512], FP8, "ya")
                nc.sync.dma_start(ya[:, 0, 0:HALF], a2a_out1[2 * i].opt())
                nc.sync.dma_start(ya[:, 1, 0:HALF], a2a_out1[2 * i + 1].opt())
                ya_t.append(ya)
            # --- batch 1 ---
            for j in range(4):
                qkv_chunk(j + 4)
                if j + 6 < NCHUNK:
                    stats_chunk(j + 6)
                prefetch(j)
                att_chunk(1, j)
            nc.gpsimd.collective_compute(
                "AllToAll", OP.bypass,
                replica_groups=[list(range(N_CORES))],
                ins=[a2a_in2.opt()], outs=[a2a_out2.opt()],
            )
            # batch-0 half of the AO projection: inputs landed with A2A#1.
            # Issued BEFORE the ya batch-1-half loads: the coarse interval
            # tracker would otherwise chain these reads behind A2A#2.
            for w in range(NCH):
                ps = _t(psq, [128, HALF], F32, "psq")
                for i in range(NCP):
                    nc.tensor.matmul(ps[:], wao_t[:, i, :, w, :],
                                     ya_t[i][:, :, 0:HALF], perf_mode=DR,
                                     start=(i == 0), stop=(i == NCP - 1))
                dst = ao_b0[w // 2][:, w % 2, 0:HALF]
                nc.vector.scalar_tensor_tensor(
                    out=dst, in0=ps[:], scalar=1.0 / (WS * YS),
                    in1=xts_t[:, w, 0:HALF], op0=OP.mult, op1=OP.add)
                if with_bias_ao:
                    nc.vector.tensor_scalar(
                        out=dst, in0=dst, scalar1=bao_t[:, w:w + 1],
                        scalar2=0.0, op0=OP.add, op1=OP.add)
            for i in range(NCP):
                nc.sync.dma_start(ya_t[i][:, 0, HALF:512],
                                  a2a_out2[2 * i].opt())
                nc.sync.dma_start(ya_t[i][:, 1, HALF:512],
                                  a2a_out2[2 * i + 1].opt())
            # preload the first FC weight groups while the A2A drains
            w3_pre = {}
            for fg in range(2):
                wt = _t(pool_w3, [128, NCH, 4, 128], BF16, "w3")
                nc.scalar.dma_start(wt[:], wfc_d[fg])
                w3_pre[fg] = wt

        # ---------- Phase D: AO proj + LN2 + MLP on merged 512 tokens ----
        with ExitStack() as es3:
            psD = es3.enter_context(tc.tile_pool(name="psD", bufs=5,
                                                 space="PSUM"))
            pool_sq = es3.enter_context(tc.tile_pool(name="sq", bufs=3))
            pool_h2 = es3.enter_context(tc.tile_pool(name="h2", bufs=NCH))
            pool_mt = es3.enter_context(tc.tile_pool(name="mt", bufs=NFB))
            pool_wm = es3.enter_context(tc.tile_pool(name="wm", bufs=3))
            pool_gb = es3.enter_context(tc.tile_pool(name="gb", bufs=3))
            pool_row2 = es3.enter_context(tc.tile_pool(name="row2", bufs=2))
            pool_bc2 = es3.enter_context(tc.tile_pool(name="bc2", bufs=2))
            pool_tmp2 = es3.enter_context(tc.tile_pool(name="tmp2", bufs=2))
            pool_ot = es3.enter_context(tc.tile_pool(name="ot", bufs=2))

            # AO projection, batch-1 half (batch 0 ran during the A2A)
            ao_pair = ao_b0
            for w in range(NCH):
                ps = _t(psD, [128, HALF], F32, "psD")
                for i in range(NCP):
                    nc.tensor.matmul(ps[:], wao_t[:, i, :, w, :],
                                     ya_t[i][:, :, HALF:512], perf_mode=DR,
                                     start=(i == 0), stop=(i == NCP - 1))
                dst = ao_pair[w // 2][:, w % 2, HALF:512]
                nc.vector.scalar_tensor_tensor(
                    out=dst, in0=ps[:], scalar=1.0 / (WS * YS),
                    in1=xts_t[:, w, HALF:512], op0=OP.mult, op1=OP.add)
                if with_bias_ao:
                    nc.vector.tensor_scalar(
                        out=dst, in0=dst, scalar1=bao_t[:, w:w + 1],
                        scalar2=0.0, op0=OP.add, op1=OP.add)
            # LN2 stats: s1 row 0, s2 row 32, single PSUM bank
            sp = _t(psD, [64, 512], F32, "psD")
            sqs = []
            for w in range(NCH):
                s = _t(pool_sq, [128, 512], BF16, "sq")
                src = ao_pair[w // 2][:, w % 2, :]
                nc.vector.tensor_tensor(s[:], src, src, OP.mult)
                sqs.append(s)
            for w in range(NCH):
                nc.tensor.matmul(sp[0:1, :], ones_bf[:],
                                 ao_pair[w // 2][:, w % 2, :],
                                 start=(w == 0), stop=(w == NCH - 1))
            for w in range(NCH):
                nc.tensor.matmul(sp[32:33, :], ones_bf[:], sqs[w][:],
                                 start=False, stop=(w == NCH - 1),
                                 skip_group_check=True)
            mur = _t(pool_row2, [1, 512], BF16, "mur")
            nc.vector.tensor_scalar(out=mur[:], in0=sp[0:1, :],
                                    scalar1=1.0 / C, scalar2=0.0,
                                    op0=OP.mult, op1=OP.add)
            varp = _t(pool_row2, [1, 512], F32, "varp2")
            nc.vector.scalar_tensor_tensor(
                out=varp[:], in0=sp[0:1, :], scalar=-1.0,
                in1=mur[:], op0=OP.mult, op1=OP.mult)
            varq = _t(pool_row2, [1, 512], F32, "varq2")
            nc.vector.tensor_tensor(varq[:], varp[:], sp[32:33, :], OP.add)
            lnv = _t(pool_row2, [1, 512], F32, "lnv2")
            nc.scalar.activation(lnv[:], varq[:], AF.Ln,
                                 bias=eps_t[0:1, :], scale=1.0 / C)
            rrow = _t(pool_row2, [1, 512], BF16, "rrow2")
            nc.scalar.activation(rrow[:], lnv[:], AF.Exp, scale=-0.5)
            mup = _t(psD, [128, 512], F32, "psD")
            nc.tensor.matmul(mup[:], onesrow_bf[:], mur[:])
            mubc = _t(pool_bc2, [128, 512], F32, "mubc")
            nc.vector.tensor_copy(mubc[:], mup[:])
            rbp = _t(psD, [128, 512], F32, "psD")
            nc.tensor.matmul(rbp[:], onesrow_bf[:], rrow[:])
            rbc2 = _t(pool_bc2, [128, 512], F32, "rbc2")
            nc.vector.tensor_copy(rbc2[:], rbp[:])
            h2 = []
            for w in range(NCH):
                tp = _t(pool_tmp2, [128, 512], F32, "tmp2")
                nc.vector.tensor_tensor(tp[:], ao_pair[w // 2][:, w % 2, :],
                                        mubc[:], OP.subtract)
                ht = _t(pool_h2, [128, 512], BF16, "h2")
                nc.vector.tensor_tensor(ht[:], tp[:], rbc2[:], OP.mult)
                h2.append(ht)
            # preload the first MP weight tiles so their transfers do not
            # queue behind the 32 gelus on the scalar queue
            wm_pre = {}
            for w in range(2):
                wt = _t(pool_wm, [128, NFB // 2, 2, 128], FP8, "wm")
                nc.scalar.dma_start(wt[:], wmp_d[w])
                wm_pre[w] = wt
            # FC + GELU (bf16, N=512)
            mt = []
            for fg in range(NFB // 4):
                if fg in w3_pre:
                    wt = w3_pre.pop(fg)
                else:
                    wt = _t(pool_w3, [128, NCH, 4, 128], BF16, "w3")
                    nc.scalar.dma_start(wt[:], wfc_d[fg])
                for fs in range(4):
                    f = 4 * fg + fs
                    ps = _t(psD, [128, 512], F32, "psD")
                    for cb in range(NCH):
                        nc.tensor.matmul(ps[:], wt[:, cb, fs, :],
                                         h2[cb][:],
                                         start=(cb == 0),
                                         stop=(cb == NCH - 1))
                    gb = _t(pool_gb, [128, 512], BF16, "gb")
                    nc.scalar.activation(gb[:], ps[:], AF.Gelu,
                                         bias=bfc_t[:, f:f + 1],
                                         scale=1.0)
                    if f % 2 == 0:
                        mt.append(_t(pool_mt, [128, 2, 512], FP8, "mt"))
                    # x4 pre-scale dodges fp8 subnormals on small gelu outputs
                    nc.vector.tensor_scalar(
                        out=mt[f // 2][:, f % 2, :], in0=gb[:],
                        scalar1=MTS, scalar2=0.0, op0=OP.mult, op1=OP.add)
            # MP + residual -> out (fp8 DoubleRow over 16 f-block pairs)
            for w in range(NCH):
                if w in wm_pre:
                    wt = wm_pre.pop(w)
                else:
                    wt = _t(pool_wm, [128, NFB // 2, 2, 128], FP8, "wm")
                    nc.scalar.dma_start(wt[:], wmp_d[w])
                ps = _t(psD, [128, 512], F32, "psD")
                for u in range(NFB // 2):
                    nc.tensor.matmul(ps[:], wt[:, u, :, :], mt[u][:],
                                     perf_mode=DR, start=(u == 0),
                                     stop=(u == NFB // 2 - 1))
                ot = _t(pool_ot, [128, 512], F32, "ot")
                nc.vector.scalar_tensor_tensor(
                    out=ot[:], in0=ps[:], scalar=1.0 / (WS * MTS),
                    in1=ao_pair[w // 2][:, w % 2, :], op0=OP.mult, op1=OP.add)
                if with_bias_mp:
                    nc.vector.tensor_scalar(
                        out=ot[:], in0=ot[:], scalar1=bmp_t[:, w:w + 1],
                        scalar2=0.0, op0=OP.add, op1=OP.add)
                nc.sync.dma_start(
                    out_d[128 * w:128 * (w + 1), :], ot[:])

    nc.compile()
    return nc


def _prep(inputs):
    """Host-side preprocessing: fold LN affines into weights, quantize."""
    f32 = np.float32
    bf16 = ml_dtypes.bfloat16
    fp8 = ml_dtypes.float8_e4m3
    x = np.asarray(inputs["x"], f32).reshape(NTOK, C)
    W_qkv = np.asarray(inputs["W_qkv"], f32)
    b_qkv = np.asarray(inputs["b_qkv"], f32)
    W_ao = np.asarray(inputs["W_ao"], f32)
    b_ao = np.asarray(inputs["b_ao"], f32)
    W_fc = np.asarray(inputs["W_fc"], f32)
    b_fc = np.asarray(inputs["b_fc"], f32)
    W_mp = np.asarray(inputs["W_mp"], f32)
    b_mp = np.asarray(inputs["b_mp"], f32)
    g1 = np.asarray(inputs["g1"], f32)
    be1 = np.asarray(inputs["be1"], f32)
    g2 = np.asarray(inputs["g2"], f32)
    be2 = np.asarray(inputs["be2"], f32)

    def q8(a):
        return np.clip(a, -240.0, 240.0).astype(fp8)

    Wq_eff = W_qkv * g1[:, None]
    bq_eff = b_qkv + be1 @ W_qkv
    bq_eff[:C] *= 1.0 / np.sqrt(D)  # 1/sqrt(D) on the Q bias only
    Wfc_eff = W_fc * g2[:, None]
    bfc_eff = b_fc + be2 @ W_fc

    xT = np.ascontiguousarray(x.T)
    # [C, NTOK] -> [NCHUNK, 128, NCP, 2, 512] (c = 256*i + 128*j + p)
    xT_q = np.ascontiguousarray(
        q8(xT).reshape(NCP, 2, 128, NCHUNK, 512).transpose(3, 2, 0, 1, 4))
    mask8 = (np.arange(128)[:, None] <= np.arange(128)[None, :]).astype(fp8)
    ident = np.eye(128, dtype=bf16)

    # [C, C] -> [128, NCP, 2, NCH, 128]
    wao_q = np.ascontiguousarray(
        q8(W_ao * WS).reshape(NCP, 2, 128, NCH, 128).transpose(2, 0, 1, 3, 4))
    # [C, F] -> [NFB//4, 128, NCH, 4, 128]
    wfc_bf = np.ascontiguousarray(
        Wfc_eff.astype(bf16).reshape(NCH, 128, NFB // 4, 4, 128).transpose(
            2, 1, 0, 3, 4))
    # [F, C] -> [NCH, 128, NFB//2, 2, 128]  (f = 256*u + 128*jj + p)
    wmp_bf = np.ascontiguousarray(
        q8(W_mp * WS).reshape(NFB // 2, 2, 128, NCH, 128).transpose(
            3, 2, 0, 1, 4))

    with_bias_qkv = bool(np.any(bq_eff != 0.0))
    with_bias_ao = bool(np.any(b_ao != 0.0))
    with_bias_mp = bool(np.any(b_mp != 0.0))

    bfc_col = np.ascontiguousarray(bfc_eff.astype(f32).reshape(NFB, 128).T)
    bmp_col = np.ascontiguousarray(b_mp.reshape(NCH, 128).T)
    bao_col = np.ascontiguousarray(b_ao.reshape(NCH, 128).T)

    in_maps = []
    for r in range(N_CORES):
        cs = 128 * r
        wq_core = np.concatenate(
            [Wq_eff[:, cs:cs + 128], Wq_eff[:, C + cs:C + cs + 128],
             Wq_eff[:, 2 * C + cs:2 * C + cs + 128]], axis=1)
        wq_q = q8(wq_core * WS)
        bq_core = np.concatenate(
            [bq_eff[cs:cs + 128], bq_eff[C + cs:C + cs + 128],
             bq_eff[2 * C + cs:2 * C + cs + 128]])
        # rank-1 mean term must cancel against the QUANTIZED weights
        wsum_core = wq_q.astype(f32).sum(axis=0)
        # [C, 384] -> [128, 3, NCP, 2, 128]
        wq_perm = np.ascontiguousarray(
            wq_q.reshape(NCP, 2, 128, 3, 128).transpose(2, 3, 0, 1, 4))
        # phase-D half-shards: 256 tokens of batch 0 + 256 of batch 1
        xts_core = np.concatenate(
            [xT[:, HALF * r:HALF * (r + 1)],
             xT[:, T + HALF * r:T + HALF * (r + 1)]], axis=1)
        xts_perm = np.ascontiguousarray(
            xts_core.reshape(NCH, 128, 2 * HALF).transpose(1, 0, 2))
        in_maps.append({
            "xT": xT_q,
            "xTs": xts_perm,
            "wqkv": wq_perm,
            "wsum": np.ascontiguousarray(wsum_core).astype(bf16).reshape(1, -1),
            "bqkv": np.ascontiguousarray(bq_core).astype(bf16).reshape(1, -1),
            "wao": wao_q,
            "bao": bao_col,
            "wfc": wfc_bf,
            "bfc": bfc_col,
            "wmp": wmp_bf,
            "bmp": bmp_col,
            "mask": mask8,
            "ident": ident,
        })
    return in_maps, with_bias_qkv, with_bias_ao, with_bias_mp


def kernel(_trace=False, _trace_kwargs=None, **inputs):
    in_maps, with_bias_qkv, with_bias_ao, with_bias_mp = _prep(inputs)
    key = ("nc", with_bias_qkv, with_bias_ao, with_bias_mp)
    if key not in _CACHE:
        _CACHE[key] = _build(with_bias_qkv, with_bias_ao, with_bias_mp)
    nc = _CACHE[key]
    res = run_bass_kernel_spmd(
        nc, in_maps, core_ids=list(range(N_CORES)),
        trace=_trace, **(_trace_kwargs or {}))
    _CACHE["last_results"] = res
    # core r output: cols 0-255 = batch-0 tokens [256r,256r+256),
    #                cols 256-511 = batch-1 tokens [256r,256r+256)
    out = np.empty((B, T, C), np.float32)
    for r in range(N_CORES):
        o = np.asarray(res.results[r]["out"])
        out[0, HALF * r:HALF * (r + 1)] = o[:, 0:HALF].T
        out[1, HALF * r:HALF * (r + 1)] = o[:, HALF:2 * HALF].T
    return out


# revision 55
# speedup vs baseline: 1.0885x; 1.0885x over previous
"""Trainium2 Bass kernel for a GPT-style transformer block (B=2, T=2048, C=1024, H=16).

Sharding: Megatron-style tensor parallelism over 8 NeuronCores.
  - Attention is head-parallel: each core computes QKV / attention for its 2 heads
    over all 4096 tokens.
  - Per batch, a small fp8 AllToAll (256 KB/core) redistributes unnormalized-exp
    attention outputs from head-sharded to token-sharded.
  - The output projection, LayerNorm2 and the MLP run token-parallel on a merged
    512-token shard per core (256 tokens of each batch) so FC/MP matmuls stream
    N=512 moving operands.

fp8 (e4m3, TRN max +-240) with DoubleRow double-pumping is used for the GEMMs
where quantization noise is cheap: QKV projection, LN stats, attention AV, and
the attention output projection.  FC and MP stay bf16 (fp8 there costs ~1.7%
rel err each -- over the 2e-2 budget).  Weights for fp8 GEMMs are pre-scaled
x16 on the host to dodge e4m3 subnormals; the descale rides for free on the
existing per-token affine multiplies (rstd/16) and stt scalars.  y ships x8.

Layouts are "transposed" [feature, token].  LN affines fold into weights on the
host; LN1 normalization folds around the QKV matmul as a per-token affine
(rank-1 wsum x nmu subtracts the mean inside the PSUM group).  rstd comes from
exp(-0.5*ln(var+eps)) so ScalarE stays inside the ln/exp table set the softmax
needs; squares run on VectorE only (no Square-table thrash).  Row stats are
broadcast across partitions with GpSimd stride-0 DMAs off a DRAM bounce;
softmax denominators ride a ones-column in the V stationary; V tiles are
PE-transposed.  Collectives and the phase-D weight prefetch issue on the DVE
queue so they never block the GpSimd broadcast queue.
"""

from contextlib import ExitStack

import numpy as np
import ml_dtypes

import concourse.bass as bass
import concourse.bacc as bacc
import concourse.mybir as mybir
import concourse.tile as tile
from concourse.bass_utils import run_bass_kernel_spmd

BF16 = mybir.dt.bfloat16
FP8 = mybir.dt.float8e4
F32 = mybir.dt.float32
AF = mybir.ActivationFunctionType
OP = mybir.AluOpType
DR = mybir.MatmulPerfMode.DoubleRow

N_CORES = 8
B, T, C, H, D = 2, 2048, 1024, 16, 64
NTOK = B * T  # 4096
F = 4 * C  # 4096
LN_EPS = 1e-5
HPC = H // N_CORES  # heads per core = 2
HALF = 256  # phase-D tokens per batch per core
NCH = C // 128  # 8 channel blocks
NCP = NCH // 2  # 4 channel pairs (DoubleRow)
NFB = F // 128  # 32 ffn blocks
NCHUNK = NTOK // 512  # 8 token chunks of 512
SB = 128  # s-block size
VWP = 80  # padded V block width (64 V + 1 ones + 15 pad; 16-aligned)
WS = 16.0  # fp8 weight pre-scale
YS = 8.0  # fp8 y pre-scale
MTS = 4.0  # fp8 gelu-output pre-scale
LOG_WS = float(np.log(WS))

_CACHE = {}

_UID = [0]


def _t(pool, shape, dtype, tag):
    _UID[0] += 1
    return pool.tile(shape, dtype, tag=tag, name=f"{tag}_{_UID[0]}")


def _pbc(ap, n):
    """Partition-broadcast AP: read `ap` n times across partitions."""
    return bass.AP(tensor=ap.tensor, offset=ap.offset,
                   ap=[[0, n]] + [list(x) for x in ap.ap])


def _build(with_bias_qkv: bool, with_bias_ao: bool,
           with_bias_mp: bool):
    nc = bacc.Bacc("TRN2", target_bir_lowering=False, debug=False,
                   num_devices=N_CORES)

    # ---- I/O ----
    xT_d = nc.dram_tensor("xT", [NCHUNK, 128, NCP, 2, 512], FP8,
                          kind="ExternalInput")
    xTs_d = nc.dram_tensor("xTs", [128, NCH, 2 * HALF], F32,
                           kind="ExternalInput")
    wqkv_d = nc.dram_tensor("wqkv", [128, 3, NCP, 2, 128], FP8,
                            kind="ExternalInput")
    wsum_d = nc.dram_tensor("wsum", [1, 3 * 128], BF16, kind="ExternalInput")
    bqkv_d = nc.dram_tensor("bqkv", [1, 3 * 128], BF16, kind="ExternalInput")
    wao_d = nc.dram_tensor("wao", [128, NCP, 2, NCH, 128], FP8,
                           kind="ExternalInput")
    bao_d = nc.dram_tensor("bao", [128, NCH], F32, kind="ExternalInput")
    wfc_d = nc.dram_tensor("wfc", [NFB // 4, 128, NCH, 4, 128], BF16,
                           kind="ExternalInput")
    bfc_d = nc.dram_tensor("bfc", [128, NFB], F32, kind="ExternalInput")
    wmp_d = nc.dram_tensor("wmp", [NCH, 128, NFB // 2, 2, 128], FP8,
                           kind="ExternalInput")
    bmp_d = nc.dram_tensor("bmp", [128, NCH], F32, kind="ExternalInput")
    mask_d = nc.dram_tensor("mask", [128, 128], FP8, kind="ExternalInput")
    ident_d = nc.dram_tensor("ident", [128, 128], BF16, kind="ExternalInput")
    out_d = nc.dram_tensor("out", [C, 2 * HALF], F32, kind="ExternalOutput")

    with tile.TileContext(nc) as tc, ExitStack() as _es:
        singles = _es.enter_context(tc.tile_pool(name="singles", bufs=1))
        dram = _es.enter_context(tc.tile_pool(name="dram", bufs=1, space="DRAM"))
        resid = _es.enter_context(tc.tile_pool(name="resid", bufs=1))
        pool_ya = _es.enter_context(tc.tile_pool(name="ya", bufs=4))
        pool_w3 = _es.enter_context(tc.tile_pool(name="w3", bufs=3))
        pool_ao = _es.enter_context(tc.tile_pool(name="ao", bufs=4))

        # DRAM bounce rows for broadcasts; A2A buffers (fp8)
        arow_d = _t(dram, [NCHUNK, 512], F32, "arow_d")
        den_d = _t(dram, [NCHUNK, HPC, 512], F32, "den_d")
        a2a_in1 = _t(dram, [N_CORES, HPC, D, HALF], FP8, "a2a_in1")
        a2a_out1 = _t(dram, [N_CORES, HPC, D, HALF], FP8, "a2a_out1")
        a2a_in2 = _t(dram, [N_CORES, HPC, D, HALF], FP8, "a2a_in2")
        a2a_out2 = _t(dram, [N_CORES, HPC, D, HALF], FP8, "a2a_out2")

        with ExitStack() as es1:
            # PSUM: psq 2 banks, psc 2x2 banks, psy 2 banks = 8
            psq = es1.enter_context(tc.tile_pool(name="psq", bufs=2,
                                                 space="PSUM"))
            pscp = es1.enter_context(tc.tile_pool(name="ps_c", bufs=2,
                                                  space="PSUM"))
            psyp = es1.enter_context(tc.tile_pool(name="ps_y", bufs=2,
                                                  space="PSUM"))
            pool_xt = es1.enter_context(tc.tile_pool(name="xt", bufs=NCHUNK))
            pool_sqx = es1.enter_context(tc.tile_pool(name="sqx", bufs=3))
            pool_row = es1.enter_context(tc.tile_pool(name="row", bufs=3))
            pool_bc = es1.enter_context(tc.tile_pool(name="bc", bufs=3))
            pool_vt = es1.enter_context(tc.tile_pool(name="vt", bufs=2))
            pool_att = es1.enter_context(tc.tile_pool(name="att", bufs=4))
            # enough yt buffers that batch-1 stores can queue behind A2A#1
            pool_yt = es1.enter_context(tc.tile_pool(name="yt", bufs=8))
            pool_qkv = es1.enter_context(tc.tile_pool(name="qkv", bufs=1))

            xt_tiles = {}

            def load_chunk(g):
                t0 = 512 * g
                xt = _t(pool_xt, [128, NCP, 2, 512], FP8, "xt")
                eng = nc.sync if g in (1, 3, 5, 7) else nc.scalar
                eng.dma_start(xt[:], xT_d[g])
                xt_tiles[g] = xt

            # chunk 0 first so PE can start ASAP, then weights, then rest
            load_chunk(0)

            ones_bf = _t(singles, [128, 1], BF16, "ones_bf")
            nc.vector.memset(ones_bf[:], 1.0)
            ones2 = _t(singles, [128, 2, 16], FP8, "ones2")
            nc.vector.memset(ones2[:], 1.0)
            onesrow_bf = _t(singles, [1, 128], BF16, "onesrow_bf")
            nc.vector.memset(onesrow_bf[:], 1.0)
            onesrow_f32 = _t(singles, [1, 64], F32, "onesrow_f32")
            nc.vector.memset(onesrow_f32[:], 1.0)
            eps_t = _t(singles, [128, 1], F32, "eps")
            nc.vector.memset(eps_t[:], LN_EPS)
            nlws_t = _t(singles, [128, 1], F32, "nlws")
            nc.vector.memset(nlws_t[:], -LOG_WS)

            ao_b0 = [_t(pool_ao, [128, 2, 512], BF16, "aop")
                     for _ in range(NCP)]
            wqkv_t = _t(resid, [128, 3, NCP, 2, 128], FP8, "wqkv")
            nc.sync.dma_start(wqkv_t[:], wqkv_d[:])
            wsum_t = _t(resid, [1, 384], BF16, "wsum")
            nc.sync.dma_start(wsum_t[:], wsum_d[:])

            load_chunk(1)

            # phase-D resident tiles; DMAs staggered into the chunk loop on
            # the Scalar queue so no single queue stalls at startup
            wao_t = _t(resid, [128, NCP, 2, NCH, 128], FP8, "wao")
            xts_t = _t(resid, [128, NCH, 2 * HALF], F32, "xts")
            bfc_t = _t(singles, [128, NFB], F32, "bfc")
            bmp_t = _t(singles, [128, NCH], F32, "bmp")
            bao_t = _t(singles, [128, NCH], F32, "bao")

            def prefetch(step):
                if step < 0:
                    return
                if step == 0:
                    nc.scalar.dma_start(wao_t[:], wao_d[:])
                elif step == 1:
                    nc.scalar.dma_start(xts_t[:], xTs_d[:])
                elif step == 2:
                    nc.scalar.dma_start(bfc_t[:], bfc_d[:])
                    nc.scalar.dma_start(bmp_t[:], bmp_d[:])
                    if with_bias_ao:
                        nc.scalar.dma_start(bao_t[:], bao_d[:])

            for g in (2, 3, 4, 6):
                load_chunk(g)

            mask_t = _t(resid, [128, 128], FP8, "mask")
            nc.sync.dma_start(mask_t[:], mask_d[:])
            ident_t = _t(resid, [128, 128], BF16, "ident")
            nc.sync.dma_start(ident_t[:], ident_d[:])
            bqkv_col_t = _t(resid, [128, 3], F32, "bqkv_col")
            if with_bias_qkv:
                nc.gpsimd.dma_start(
                    bqkv_col_t[:],
                    bqkv_d.ap()[0, :].rearrange("(o p) -> p o", p=128))

            # attention operand tiles (resident, written per chunk)
            qT_t = _t(pool_qkv, [128, NTOK], BF16, "qT")
            kT_t = _t(pool_qkv, [128, NTOK], BF16, "kT")
            vT_t = _t(pool_qkv, [128, NTOK], BF16, "vT")

            sq_tiles = {}

            def squares(g):
                xt = xt_tiles[g]
                sq = _t(pool_sqx, [128, NCP, 2, 512], FP8, "sqx")
                nc.vector.tensor_tensor(sq[:, 0:2], xt[:, 0:2], xt[:, 0:2],
                                        OP.mult)
                nc.vector.tensor_tensor(sq[:, 2:4], xt[:, 2:4], xt[:, 2:4],
                                        OP.mult)
                sq_tiles[g] = sq

            ab_tiles = {}

            def stats_chunk(g):
                """s1 = sum_c x (row 0), s2 = sum_c x^2 (row 32) in one PSUM
                bank via DoubleRow fp8; then the row chain + abc broadcast."""
                xt = xt_tiles[g]
                sq = sq_tiles.pop(g)
                sp = _t(psq, [64, 512], F32, "psq")
                for i in range(NCP):
                    nc.tensor.matmul(sp[0:1, :], ones2[:, :, 0:1],
                                     xt[:, i, :, :], perf_mode=DR,
                                     start=(i == 0), stop=(i == NCP - 1))
                # DoubleRow cannot target PSUM base partition 32, so the
                # s2 chain runs as plain fp8 matmuls (bf16 rate)
                for i in range(NCP):
                    for jj in range(2):
                        nc.tensor.matmul(sp[32:33, :], ones2[:, 0, 0:1],
                                         sq[:, i, jj, :],
                                         start=False,
                                         stop=(i == NCP - 1 and jj == 1),
                                         skip_group_check=True)
                nmu = _t(pool_row, [1, 512], BF16, "nmu")
                nc.vector.tensor_scalar(out=nmu[:], in0=sp[0:1, :],
                                        scalar1=-1.0 / C, scalar2=0.0,
                                        op0=OP.mult, op1=OP.add)
                # varp = s2 - s1^2/C  (= var*C); squares on DVE, no Act table
                # (s1 * nmu = -s1^2/C; only one PSUM operand allowed)
                varp = _t(pool_row, [1, 512], F32, "varp")
                nc.vector.scalar_tensor_tensor(
                    out=varp[:], in0=sp[0:1, :], scalar=1.0,
                    in1=nmu[:], op0=OP.mult, op1=OP.mult)
                varq = _t(pool_row, [1, 512], F32, "varq")
                nc.vector.tensor_tensor(varq[:], varp[:], sp[32:33, :], OP.add)
                # rstd/WS = exp(-0.5*ln(varp/C + eps) - ln WS)
                lnv = _t(pool_row, [1, 512], F32, "lnv")
                nc.scalar.activation(lnv[:], varq[:], AF.Ln,
                                     bias=eps_t[0:1, :], scale=1.0 / C)
                arow = _t(pool_row, [1, 512], F32, "arow")
                nc.scalar.activation(arow[:], lnv[:], AF.Exp, scale=-0.5,
                                     bias=nlws_t[0:1, :])
                nc.sync.dma_start(arow_d[g], arow[0:1, :])
                abc = _t(pool_bc, [128, 512], F32, "abc")
                nc.sync.dma_start(abc[:], _pbc(arow_d[g], 128))
                ab_tiles[g] = (abc, nmu)

            def qkv_piece(g, o, dst_t):
                """One projection (Q, K or V) of chunk g -- a ~1us PE filler
                sized to hide one softmax-exp latency window."""
                def run():
                    t0 = 512 * g
                    xt = xt_tiles[g]
                    abc, nmu = ab_tiles[g]
                    ps = _t(psq, [128, 512], F32, "psq")
                    for i in range(NCP):
                        nc.tensor.matmul(ps[:], wqkv_t[:, o, i, :, :],
                                         xt[:, i, :, :], perf_mode=DR,
                                         start=(i == 0), stop=False)
                    # rank-1 term: wsum (x) nmu subtracts the mean
                    nc.tensor.matmul(ps[:], wsum_t[:, 128 * o:128 * (o + 1)],
                                     nmu[:], start=False, stop=True,
                                     skip_group_check=True)
                    dst = dst_t[:, t0:t0 + 512]
                    if o == 0:  # fold 1/sqrt(D) into Q
                        nc.vector.scalar_tensor_tensor(
                            out=dst, in0=ps[:], scalar=0.125, in1=abc[:],
                            op0=OP.mult, op1=OP.mult)
                    else:
                        nc.vector.tensor_tensor(dst, ps[:], abc[:], OP.mult)
                    if with_bias_qkv:
                        nc.vector.tensor_scalar(
                            out=dst, in0=dst,
                            scalar1=bqkv_col_t[:, o:o + 1], scalar2=0.0,
                            op0=OP.add, op1=OP.add)
                    if o == 2:
                        xt_tiles.pop(g)
                        ab_tiles.pop(g)
                return run

            def qkv_fillers(g):
                return [qkv_piece(g, 0, qT_t), qkv_piece(g, 1, kT_t),
                        qkv_piece(g, 2, vT_t)]

            def qkv_chunk(g):
                for f in qkv_fillers(g):
                    f()

            vt = {}

            def att_chunk(b, j, fillers=()):
                fillers = list(fillers)
                g = 4 * b + j
                t0 = 512 * g
                if j == 0:
                    for h in range(HPC):
                        v = _t(pool_vt, [128, T // SB // 2, 2, VWP], FP8,
                               f"vt{h}")
                        nc.vector.memset(v[:, :, :, D:D + 1], 1.0)
                        nc.vector.memset(v[:, :, :, D + 1:VWP], 0.0)
                        vt[b, h] = v
                # transpose V for the newly available s-blocks
                for i in range(4 * j, 4 * j + 4):
                    s0 = 2048 * b + SB * i
                    for h in range(HPC):
                        pst = _t(psq, [128, D], BF16, "psq")
                        nc.tensor.transpose(
                            pst[:],
                            vT_t[64 * h:64 * (h + 1), s0:s0 + SB],
                            ident_t[64 * h:64 * (h + 1),
                                    64 * h:64 * (h + 1)])
                        nc.vector.tensor_copy(
                            vt[b, h][:, i // 2, i % 2, 0:D], pst[:])
                psy = [_t(psyp, [VWP, 512], F32, "psy") for h in range(HPC)]
                npair = 2 * j + 2
                pend_av = None
                for p in range(npair):
                    at = _t(pool_att, [128, 2, HPC, 512], FP8, "att")
                    for par in range(2):
                        i = 2 * p + par
                        m = i - 4 * j  # >= 0 on diagonal blocks
                        f0 = 128 * m if m >= 0 else 0
                        s0 = 2048 * b + SB * i
                        psc = _t(pscp, [128, HPC, 512], F32, "psc")
                        for h in range(HPC):
                            nc.tensor.matmul(
                                psc[:, h, 0:512 - f0],
                                kT_t[64 * h:64 * (h + 1), s0:s0 + SB],
                                qT_t[64 * h:64 * (h + 1), t0 + f0:t0 + 512],
                                tile_position=(64 * h, 0),
                            )
                        nc.scalar.activation(at[:, par, :, f0:512],
                                             psc[:, :, 0:512 - f0], AF.Exp)
                        if m >= 0:  # diagonal: mask boundary block
                            for h in range(HPC):
                                nc.vector.tensor_tensor(
                                    at[:, par, h, f0:f0 + 128],
                                    at[:, par, h, f0:f0 + 128],
                                    mask_t[:], OP.mult)
                    # zero the invalid wedge of the second diagonal block
                    pf0 = 256 if p == 2 * j + 1 else 0
                    mlast = 2 * p + 1 - 4 * j
                    if mlast >= 0:
                        w0 = 128 * (mlast - 1) if mlast > 0 else 0
                        nc.vector.memset(at[:, 1, :, w0:128 * mlast], 0.0)
                    # AV trails by one pair so the next pair's QK matmuls
                    # hide the softmax-exp latency instead of the PE stalling
                    if pend_av is not None:
                        pend_av()

                    def mk_av(p=p, at=at, pf0=pf0):
                        def run():
                            for h in range(HPC):
                                nc.tensor.matmul(
                                    psy[h][:, pf0:512],
                                    vt[b, h][:, p, :, :],
                                    at[:, :, h, pf0:512],
                                    perf_mode=DR,
                                    start=(p == 0), stop=(p == npair - 1),
                                    skip_group_check=True)
                        return run
                    pend_av = mk_av()
                pend_av()
                # normalize y on the producer, then ship 256-col halves
                a2a_in = a2a_in1 if b == 0 else a2a_in2
                rr = _t(pool_row, [VWP, HPC, 512], F32, "rr")
                for h in range(HPC):
                    nc.vector.tensor_copy(rr[0:1, h, :],
                                          psy[h][D:D + 1, :])
                    if not (b == 1 and j == 3):
                        nc.sync.dma_start(den_d[g, h], rr[0:1, h, :])
                d0 = 2 * (g % 4)
                for h in range(HPC):
                    if b == 1 and j == 3:
                        # last chunk: PE-broadcast the denominator so the
                        # A2A#2 entry skips the DRAM bounce roundtrip
                        rbp = _t(psq, [64, 512], F32, "psq")
                        nc.tensor.matmul(rbp[:], onesrow_f32[:],
                                         rr[0:1, h, :])
                        rec = _t(pool_bc, [64, 512], F32, "rec")
                        nc.vector.reciprocal_approx_fast(rec[:], rbp[:])
                    else:
                        rbc = _t(pool_bc, [64, 512], F32, "rbc")
                        nc.sync.dma_start(rbc[:], _pbc(den_d[g, h], 64))
                        rec = _t(pool_bc, [64, 512], F32, "rec")
                        nc.vector.reciprocal_approx_fast(rec[:], rbc[:])
                    yt = _t(pool_yt, [64, 512], FP8, "yt")
                    nc.vector.scalar_tensor_tensor(
                        out=yt[:], in0=psy[h][0:D, :], scalar=YS,
                        in1=rec[:], op0=OP.mult, op1=OP.mult)
                    nc.sync.dma_start(a2a_in[d0, h].opt(), yt[:, 0:HALF])
                    nc.sync.dma_start(a2a_in[d0 + 1, h].opt(),
                                      yt[:, HALF:512])

            # --- batch 0: stats run TWO chunks ahead of qkv so the
            # abc DRAM-bounce broadcast never gates the qkv post-multiply ---
            squares(0)
            stats_chunk(0)
            load_chunk(5)
            load_chunk(7)
            squares(1)
            stats_chunk(1)
            squares(2)
            for j in range(4):
                qkv_chunk(j)
                stats_chunk(j + 2)
                squares(j + 3)
                if j == 3:
                    squares(7)
                att_chunk(0, j)
            nc.gpsimd.collective_compute(
                "AllToAll", OP.bypass,
                replica_groups=[list(range(N_CORES))],
                ins=[a2a_in1.opt()], outs=[a2a_out1.opt()],
            )
            # start loading the batch-0 halves of the ya pair tiles ASAP
            ya_t = []
            for i in range(NCP):
                ya = _t(pool_ya, [128, 2, 512], FP8, "ya")
                nc.sync.dma_start(ya[:, 0, 0:HALF], a2a_out1[2 * i].opt())
                nc.sync.dma_start(ya[:, 1, 0:HALF], a2a_out1[2 * i + 1].opt())
                ya_t.append(ya)
            # --- batch 1 ---
            for j in range(4):
                qkv_chunk(j + 4)
                if j + 6 < NCHUNK:
                    stats_chunk(j + 6)
                prefetch(j)
                att_chunk(1, j)
            nc.gpsimd.collective_compute(
                "AllToAll", OP.bypass,
                replica_groups=[list(range(N_CORES))],
                ins=[a2a_in2.opt()], outs=[a2a_out2.opt()],
            )
            # batch-0 half of the AO projection: inputs landed with A2A#1.
            # Issued BEFORE the ya batch-1-half loads: the coarse interval
            # tracker would otherwise chain these reads behind A2A#2.
            for w in range(NCH):
                ps = _t(psq, [128, HALF], F32, "psq")
                for i in range(NCP):
                    nc.tensor.matmul(ps[:], wao_t[:, i, :, w, :],
                                     ya_t[i][:, :, 0:HALF], perf_mode=DR,
                                     start=(i == 0), stop=(i == NCP - 1))
                dst = ao_b0[w // 2][:, w % 2, 0:HALF]
                nc.vector.scalar_tensor_tensor(
                    out=dst, in0=ps[:], scalar=1.0 / (WS * YS),
                    in1=xts_t[:, w, 0:HALF], op0=OP.mult, op1=OP.add)
                if with_bias_ao:
                    nc.vector.tensor_scalar(
                        out=dst, in0=dst, scalar1=bao_t[:, w:w + 1],
                        scalar2=0.0, op0=OP.add, op1=OP.add)
            for i in range(NCP):
                nc.sync.dma_start(ya_t[i][:, 0, HALF:512],
                                  a2a_out2[2 * i].opt())
                nc.sync.dma_start(ya_t[i][:, 1, HALF:512],
                                  a2a_out2[2 * i + 1].opt())
            # preload the first FC weight groups while the A2A drains
            w3_pre = {}
            for fg in range(2):
                wt = _t(pool_w3, [128, NCH, 4, 128], BF16, "w3")
                nc.scalar.dma_start(wt[:], wfc_d[fg])
                w3_pre[fg] = wt

        # ---------- Phase D: AO proj + LN2 + MLP on merged 512 tokens ----
        with ExitStack() as es3:
            psD = es3.enter_context(tc.tile_pool(name="psD", bufs=5,
                                                 space="PSUM"))
            pool_sq = es3.enter_context(tc.tile_pool(name="sq", bufs=3))
            pool_h2 = es3.enter_context(tc.tile_pool(name="h2", bufs=NCH))
            pool_mt = es3.enter_context(tc.tile_pool(name="mt", bufs=NFB))
            pool_wm = es3.enter_context(tc.tile_pool(name="wm", bufs=3))
            pool_gb = es3.enter_context(tc.tile_pool(name="gb", bufs=3))
            pool_row2 = es3.enter_context(tc.tile_pool(name="row2", bufs=2))
            pool_bc2 = es3.enter_context(tc.tile_pool(name="bc2", bufs=2))
            pool_tmp2 = es3.enter_context(tc.tile_pool(name="tmp2", bufs=2))
            pool_ot = es3.enter_context(tc.tile_pool(name="ot", bufs=2))

            # AO projection, batch-1 half (batch 0 ran during the A2A)
            ao_pair = ao_b0
            for w in range(NCH):
                ps = _t(psD, [128, HALF], F32, "psD")
                for i in range(NCP):
                    nc.tensor.matmul(ps[:], wao_t[:, i, :, w, :],
                                     ya_t[i][:, :, HALF:512], perf_mode=DR,
                                     start=(i == 0), stop=(i == NCP - 1))
                dst = ao_pair[w // 2][:, w % 2, HALF:512]
                nc.vector.scalar_tensor_tensor(
                    out=dst, in0=ps[:], scalar=1.0 / (WS * YS),
                    in1=xts_t[:, w, HALF:512], op0=OP.mult, op1=OP.add)
                if with_bias_ao:
                    nc.vector.tensor_scalar(
                        out=dst, in0=dst, scalar1=bao_t[:, w:w + 1],
                        scalar2=0.0, op0=OP.add, op1=OP.add)
            # LN2 stats: s1 row 0, s2 row 32, single PSUM bank
            sp = _t(psD, [64, 512], F32, "psD")
            sqs = []
            for w in range(NCH):
                s = _t(pool_sq, [128, 512], BF16, "sq")
                src = ao_pair[w // 2][:, w % 2, :]
                nc.vector.tensor_tensor(s[:], src, src, OP.mult)
                sqs.append(s)
            for w in range(NCH):
                nc.tensor.matmul(sp[0:1, :], ones_bf[:],
                                 ao_pair[w // 2][:, w % 2, :],
                                 start=(w == 0), stop=(w == NCH - 1))
            for w in range(NCH):
                nc.tensor.matmul(sp[32:33, :], ones_bf[:], sqs[w][:],
                                 start=False, stop=(w == NCH - 1),
                                 skip_group_check=True)
            mur = _t(pool_row2, [1, 512], BF16, "mur")
            nc.vector.tensor_scalar(out=mur[:], in0=sp[0:1, :],
                                    scalar1=1.0 / C, scalar2=0.0,
                                    op0=OP.mult, op1=OP.add)
            varp = _t(pool_row2, [1, 512], F32, "varp2")
            nc.vector.scalar_tensor_tensor(
                out=varp[:], in0=sp[0:1, :], scalar=-1.0,
                in1=mur[:], op0=OP.mult, op1=OP.mult)
            varq = _t(pool_row2, [1, 512], F32, "varq2")
            nc.vector.tensor_tensor(varq[:], varp[:], sp[32:33, :], OP.add)
            lnv = _t(pool_row2, [1, 512], F32, "lnv2")
            nc.scalar.activation(lnv[:], varq[:], AF.Ln,
                                 bias=eps_t[0:1, :], scale=1.0 / C)
            rrow = _t(pool_row2, [1, 512], BF16, "rrow2")
            nc.scalar.activation(rrow[:], lnv[:], AF.Exp, scale=-0.5)
            mup = _t(psD, [128, 512], F32, "psD")
            nc.tensor.matmul(mup[:], onesrow_bf[:], mur[:])
            mubc = _t(pool_bc2, [128, 512], F32, "mubc")
            nc.vector.tensor_copy(mubc[:], mup[:])
            rbp = _t(psD, [128, 512], F32, "psD")
            nc.tensor.matmul(rbp[:], onesrow_bf[:], rrow[:])
            rbc2 = _t(pool_bc2, [128, 512], F32, "rbc2")
            nc.vector.tensor_copy(rbc2[:], rbp[:])
            h2 = []
            for w in range(NCH):
                tp = _t(pool_tmp2, [128, 512], F32, "tmp2")
                nc.vector.tensor_tensor(tp[:], ao_pair[w // 2][:, w % 2, :],
                                        mubc[:], OP.subtract)
                ht = _t(pool_h2, [128, 512], BF16, "h2")
                nc.vector.tensor_tensor(ht[:], tp[:], rbc2[:], OP.mult)
                h2.append(ht)
            # preload the first MP weight tiles so their transfers do not
            # queue behind the 32 gelus on the scalar queue
            wm_pre = {}
            for w in range(2):
                wt = _t(pool_wm, [128, NFB // 2, 2, 128], FP8, "wm")
                nc.scalar.dma_start(wt[:], wmp_d[w])
                wm_pre[w] = wt
            # FC + GELU (bf16, N=512)
            mt = []
            for fg in range(NFB // 4):
                if fg in w3_pre:
                    wt = w3_pre.pop(fg)
                else:
                    wt = _t(pool_w3, [128, NCH, 4, 128], BF16, "w3")
                    nc.scalar.dma_start(wt[:], wfc_d[fg])
                for fs in range(4):
                    f = 4 * fg + fs
                    ps = _t(psD, [128, 512], F32, "psD")
                    for cb in range(NCH):
                        nc.tensor.matmul(ps[:], wt[:, cb, fs, :],
                                         h2[cb][:],
                                         start=(cb == 0),
                                         stop=(cb == NCH - 1))
                    gb = _t(pool_gb, [128, 512], BF16, "gb")
                    nc.scalar.activation(gb[:], ps[:], AF.Gelu,
                                         bias=bfc_t[:, f:f + 1],
                                         scale=1.0)
                    if f % 2 == 0:
                        mt.append(_t(pool_mt, [128, 2, 512], FP8, "mt"))
                    # x4 pre-scale dodges fp8 subnormals on small gelu outputs
                    nc.vector.tensor_scalar(
                        out=mt[f // 2][:, f % 2, :], in0=gb[:],
                        scalar1=MTS, scalar2=0.0, op0=OP.mult, op1=OP.add)
            # MP + residual -> out (fp8 DoubleRow over 16 f-block pairs)
            for w in range(NCH):
                if w in wm_pre:
                    wt = wm_pre.pop(w)
                else:
                    wt = _t(pool_wm, [128, NFB // 2, 2, 128], FP8, "wm")
                    nc.scalar.dma_start(wt[:], wmp_d[w])
                ps = _t(psD, [128, 512], F32, "psD")
                for u in range(NFB // 2):
                    nc.tensor.matmul(ps[:], wt[:, u, :, :], mt[u][:],
                                     perf_mode=DR, start=(u == 0),
                                     stop=(u == NFB // 2 - 1))
                ot = _t(pool_ot, [128, 512], F32, "ot")
                nc.vector.scalar_tensor_tensor(
                    out=ot[:], in0=ps[:], scalar=1.0 / (WS * MTS),
                    in1=ao_pair[w // 2][:, w % 2, :], op0=OP.mult, op1=OP.add)
                if with_bias_mp:
                    nc.vector.tensor_scalar(
                        out=ot[:], in0=ot[:], scalar1=bmp_t[:, w:w + 1],
                        scalar2=0.0, op0=OP.add, op1=OP.add)
                nc.sync.dma_start(
                    out_d[128 * w:128 * (w + 1), :], ot[:])

    nc.compile()
    return nc


def _prep(inputs):
    """Host-side preprocessing: fold LN affines into weights, quantize."""
    f32 = np.float32
    bf16 = ml_dtypes.bfloat16
    fp8 = ml_dtypes.float8_e4m3
    x = np.asarray(inputs["x"], f32).reshape(NTOK, C)
    W_qkv = np.asarray(inputs["W_qkv"], f32)
    b_qkv = np.asarray(inputs["b_qkv"], f32)
    W_ao = np.asarray(inputs["W_ao"], f32)
    b_ao = np.asarray(inputs["b_ao"], f32)
    W_fc = np.asarray(inputs["W_fc"], f32)
    b_fc = np.asarray(inputs["b_fc"], f32)
    W_mp = np.asarray(inputs["W_mp"], f32)
    b_mp = np.asarray(inputs["b_mp"], f32)
    g1 = np.asarray(inputs["g1"], f32)
    be1 = np.asarray(inputs["be1"], f32)
    g2 = np.asarray(inputs["g2"], f32)
    be2 = np.asarray(inputs["be2"], f32)

    def q8(a):
        return np.clip(a, -240.0, 240.0).astype(fp8)

    Wq_eff = W_qkv * g1[:, None]
    bq_eff = b_qkv + be1 @ W_qkv
    bq_eff[:C] *= 1.0 / np.sqrt(D)  # 1/sqrt(D) on the Q bias only
    Wfc_eff = W_fc * g2[:, None]
    bfc_eff = b_fc + be2 @ W_fc

    xT = np.ascontiguousarray(x.T)
    # [C, NTOK] -> [NCHUNK, 128, NCP, 2, 512] (c = 256*i + 128*j + p)
    xT_q = np.ascontiguousarray(
        q8(xT).reshape(NCP, 2, 128, NCHUNK, 512).transpose(3, 2, 0, 1, 4))
    mask8 = (np.arange(128)[:, None] <= np.arange(128)[None, :]).astype(fp8)
    ident = np.eye(128, dtype=bf16)

    # [C, C] -> [128, NCP, 2, NCH, 128]
    wao_q = np.ascontiguousarray(
        q8(W_ao * WS).reshape(NCP, 2, 128, NCH, 128).transpose(2, 0, 1, 3, 4))
    # [C, F] -> [NFB//4, 128, NCH, 4, 128]
    wfc_bf = np.ascontiguousarray(
        Wfc_eff.astype(bf16).reshape(NCH, 128, NFB // 4, 4, 128).transpose(
            2, 1, 0, 3, 4))
    # [F, C] -> [NCH, 128, NFB//2, 2, 128]  (f = 256*u + 128*jj + p)
    wmp_bf = np.ascontiguousarray(
        q8(W_mp * WS).reshape(NFB // 2, 2, 128, NCH, 128).transpose(
            3, 2, 0, 1, 4))

    with_bias_qkv = bool(np.any(bq_eff != 0.0))
    with_bias_ao = bool(np.any(b_ao != 0.0))
    with_bias_mp = bool(np.any(b_mp != 0.0))

    bfc_col = np.ascontiguousarray(bfc_eff.astype(f32).reshape(NFB, 128).T)
    bmp_col = np.ascontiguousarray(b_mp.reshape(NCH, 128).T)
    bao_col = np.ascontiguousarray(b_ao.reshape(NCH, 128).T)

    in_maps = []
    for r in range(N_CORES):
        cs = 128 * r
        wq_core = np.concatenate(
            [Wq_eff[:, cs:cs + 128], Wq_eff[:, C + cs:C + cs + 128],
             Wq_eff[:, 2 * C + cs:2 * C + cs + 128]], axis=1)
        wq_q = q8(wq_core * WS)
        bq_core = np.concatenate(
            [bq_eff[cs:cs + 128], bq_eff[C + cs:C + cs + 128],
             bq_eff[2 * C + cs:2 * C + cs + 128]])
        # rank-1 mean term must cancel against the QUANTIZED weights
        wsum_core = wq_q.astype(f32).sum(axis=0)
        # [C, 384] -> [128, 3, NCP, 2, 128]
        wq_perm = np.ascontiguousarray(
            wq_q.reshape(NCP, 2, 128, 3, 128).transpose(2, 3, 0, 1, 4))
        # phase-D half-shards: 256 tokens of batch 0 + 256 of batch 1
        xts_core = np.concatenate(
            [xT[:, HALF * r:HALF * (r + 1)],
             xT[:, T + HALF * r:T + HALF * (r + 1)]], axis=1)
        xts_perm = np.ascontiguousarray(
            xts_core.reshape(NCH, 128, 2 * HALF).transpose(1, 0, 2))
        in_maps.append({
            "xT": xT_q,
            "xTs": xts_perm,
            "wqkv": wq_perm,
            "wsum": np.ascontiguousarray(wsum_core).astype(bf16).reshape(1, -1),
            "bqkv": np.ascontiguousarray(bq_core).astype(bf16).reshape(1, -1),
            "wao": wao_q,
            "bao": bao_col,
            "wfc": wfc_bf,
            "bfc": bfc_col,
            "wmp": wmp_bf,
            "bmp": bmp_col,
            "mask": mask8,
            "ident": ident,
        })
    return in_maps, with_bias_qkv, with_bias_ao, with_bias_mp


def kernel(_trace=False, _trace_kwargs=None, **inputs):
    in_maps, with_bias_qkv, with_bias_ao, with_bias_mp = _prep(inputs)
    key = ("nc", with_bias_qkv, with_bias_ao, with_bias_mp)
    if key not in _CACHE:
        _CACHE[key] = _build(with_bias_qkv, with_bias_ao, with_bias_mp)
    nc = _CACHE[key]
    res = run_bass_kernel_spmd(
        nc, in_maps, core_ids=list(range(N_CORES)),
        trace=_trace, **(_trace_kwargs or {}))
    _CACHE["last_results"] = res
    # core r output: cols 0-255 = batch-0 tokens [256r,256r+256),
    #                cols 256-511 = batch-1 tokens [256r,256r+256)
    out = np.empty((B, T, C), np.float32)
    for r in range(N_CORES):
        o = np.asarray(res.results[r]["out"])
        out[0, HALF * r:HALF * (r + 1)] = o[:, 0:HALF].T
        out[1, HALF * r:HALF * (r + 1)] = o[:, HALF:2 * HALF].T
    return out
